# revision 1
# baseline (speedup 1.0000x reference)
"""Trainium2 Bass kernel for the Memoroid linear-recurrence block.

Math (per batch b, fp32):
    a = sigmoid(x @ W_a + b_a)          [T, D]
    bm = x @ W_b                        [T, D]
    h_t = a_t * h_{t-1} + bm_t          (h_{-1} = h0, scan over t)
    y = gelu_tanh(h) @ W_y + x @ W_skip [T, D]
Returns (h, y).

Strategy: data-parallel over batch (8 sequences -> 8 cores). Per core,
work in transposed layout [d, t] so the recurrence maps onto the DVE
tensor_tensor_scan instruction (state = a*state + b along the free dim).
Matmuls run as float32r (fp32 bits, fast PE mode). x is transposed into
[d_in, t] tiles with PE transposes; y is computed back in natural [t, d]
orientation directly (lhsT = gelu(h)^T tiles), h is PE-transposed back.
"""

import sys

for _p in ("/opt/trn_rl_repo",):
    if _p not in sys.path:
        sys.path.insert(0, _p)

from contextlib import ExitStack

import numpy as np

import concourse.bass as bass
import concourse.bacc as bacc
import concourse.mybir as mybir
from concourse import tile
from concourse.bass_utils import run_bass_kernel_spmd
from concourse.masks import make_identity

B, T, D = 8, 4096, 1024
P = 128
KT = D // P            # 8 partition tiles along any d-dimension
TC = 256               # time-chunk length (scan tile free dim)
NCHUNK = T // TC       # 16
TSUB = TC // P         # 2  (128-row subtiles per chunk)
NO = D // 512          # 2  (512-wide output column chunks)

f32 = mybir.dt.float32
f32r = mybir.dt.float32r

_CACHE = {}


def _build(repeat=1):
    nc = bacc.Bacc()

    x_d = nc.declare_dram_parameter("x", [T, D], f32, False)
    h0_d = nc.declare_dram_parameter("h0", [D], f32, False)
    wa_d = nc.declare_dram_parameter("wa", [D, D], f32r, False)
    ba_d = nc.declare_dram_parameter("ba", [D], f32, False)
    wb_d = nc.declare_dram_parameter("wb", [D, D], f32r, False)
    wy_d = nc.declare_dram_parameter("wy", [D, D], f32r, False)
    ws_d = nc.declare_dram_parameter("ws", [D, D], f32r, False)
    h_d = nc.declare_dram_parameter("h_out", [T, D], f32, True)
    y_d = nc.declare_dram_parameter("y_out", [T, D], f32, True)

    AF = mybir.ActivationFunctionType
    ALU = mybir.AluOpType

    with tile.TileContext(nc) as tc, ExitStack() as ctx:
        wpool = ctx.enter_context(tc.tile_pool(name="weights", bufs=1))
        const_pool = ctx.enter_context(tc.tile_pool(name="const", bufs=1))
        xn_pool = ctx.enter_context(tc.tile_pool(name="xn", bufs=2))
        xt_pool = ctx.enter_context(tc.tile_pool(name="xt", bufs=2))
        sc_pool = ctx.enter_context(tc.tile_pool(name="scan", bufs=2))
        st_pool = ctx.enter_context(tc.tile_pool(name="stage", bufs=1))
        ps_pose = ctx.enter_context(tc.tile_pool(name="pose", bufs=2, space="PSUM"))
        ps_ab = ctx.enter_context(tc.tile_pool(name="ab", bufs=3, space="PSUM"))
        ps_y = ctx.enter_context(tc.tile_pool(name="ypsum", bufs=3, space="PSUM"))

        ident = const_pool.tile([P, P], f32, name="ident")
        make_identity(nc, ident[:])

        # chunk-0 x tiles are the PE's first dependency (transposes) -> DMA
        # them before any weight traffic.
        xn0 = []
        for ts in range(TSUB):
            xt_ = xn_pool.tile([P, D], f32, tag="xn", name=f"xn0_{ts}")
            nc.sync.dma_start(xt_[:], x_d[ts * P : (ts + 1) * P, :])
            xn0.append(xt_)

        # --- persistent weights: 8 partition-tiles of [128, 1024] each.
        # wa/wb are needed by chunk 0's matmuls -> load them first; wy/ws
        # are first read in phase B (one chunk later) -> defer their DMAs
        # until after chunk 0 is emitted so they don't delay the PE start.
        wa_sb, wb_sb, wy_sb, ws_sb = [], [], [], []
        for k in range(KT):
            for lst, dram, nm in ((wa_sb, wa_d, "wa"), (wb_sb, wb_d, "wb")):
                t_ = wpool.tile([P, D], f32r, tag=f"{nm}{k}", name=f"{nm}{k}")
                nc.sync.dma_start(t_[:], dram[k * P : (k + 1) * P, :])
                lst.append(t_)

        def load_late_weights():
            for k in range(KT):
                for lst, dram, nm in ((wy_sb, wy_d, "wy"), (ws_sb, ws_d, "ws")):
                    t_ = wpool.tile([P, D], f32r, tag=f"{nm}{k}", name=f"{nm}{k}")
                    nc.sync.dma_start(t_[:], dram[k * P : (k + 1) * P, :])
                    lst.append(t_)

        ba_sb, h0_sb = [], []
        for j in range(KT):
            bt = const_pool.tile([P, 1], f32, tag=f"ba{j}", name=f"ba{j}")
            nc.sync.dma_start(bt[:], ba_d[j * P : (j + 1) * P].unsqueeze(1))
            # sigmoid(z) is computed as 0.5 + 0.5*tanh(z/2) so every ACT op
            # (Tanh/Gelu_apprx_tanh/Copy) shares one activation table ->
            # no per-op table reloads. Pre-halve the bias for the tanh form.
            bh = const_pool.tile([P, 1], f32, tag=f"bah{j}", name=f"bah{j}")
            nc.scalar.mul(bh[:], bt[:], 0.5)
            ba_sb.append(bh)
            ht = const_pool.tile([P, 1], f32, tag=f"h0{j}", name=f"h0{j}")
            nc.sync.dma_start(ht[:], h0_d[j * P : (j + 1) * P].unsqueeze(1))
            h0_sb.append(ht)

        # state carried across the chunk loop (pair tiles: jp covers j=2*jp,2*jp+1)
        NP = KT // 2            # 4 j-pairs
        hT_prev = [None] * NP   # previous chunk's hT pair tiles (carry source)
        pend = None             # (xT, gT pairs, hT pairs) of prev chunk

        for rep, c in [(r, c) for r in range(repeat) for c in range(NCHUNK + 1)]:
            if c < NCHUNK:
                t0 = c * TC
                # --- load x chunk (natural layout; chunk 0 preloaded) ---
                if c == 0 and rep == 0:
                    xn = xn0
                else:
                    xn = []
                    for ts in range(TSUB):
                        xt_ = xn_pool.tile([P, D], f32, tag="xn", name=f"xn{rep}_{c}_{ts}")
                        nc.sync.dma_start(
                            xt_[:], x_d[t0 + ts * P : t0 + (ts + 1) * P, :]
                        )
                        xn.append(xt_)

                # --- transpose x -> xT [128 d_in, KT*TC] (k-major free dim) ---
                xT = xt_pool.tile([P, KT * TC], f32r, tag="xT", name=f"xT{rep}_{c}")
                xT3 = xT[:].rearrange("p (k t) -> p k t", k=KT)
                for ts in range(TSUB):
                    for k4 in range(0, KT, 4):
                        pp = ps_pose.tile([P, 4 * P], f32, tag="pose", name=f"xp{rep}_{c}_{ts}_{k4}")
                        for i in range(4):
                            k = k4 + i
                            nc.tensor.transpose(
                                pp[:, i * P : (i + 1) * P],
                                xn[ts][:, k * P : (k + 1) * P],
                                ident[:],
                            )
                        nc.scalar.copy(
                            xT3[:, k4 : k4 + 4, ts * P : (ts + 1) * P],
                            pp[:].rearrange("p (i t) -> p i t", i=4),
                        )

            if c >= 1:
                # --- phase B for chunk c-1: y matmuls + stores + h transpose ---
                xT_p, gT_p, hT_p = pend
                t0p = (c - 1) * TC
                for ts in range(TSUB):
                    for o in range(NO):
                        psY = ps_y.tile([P, 512], f32, tag="y", name=f"psY{rep}_{c-1}_{ts}_{o}")
                        for j in range(KT):
                            nc.tensor.matmul(
                                psY[:],
                                gT_p[j // 2][:, (j % 2) * TC + ts * P : (j % 2) * TC + (ts + 1) * P],
                                wy_sb[j][:, o * 512 : (o + 1) * 512],
                                start=(j == 0),
                                stop=False,
                            )
                        for k in range(KT):
                            nc.tensor.matmul(
                                psY[:],
                                xT_p[:, k * TC + ts * P : k * TC + (ts + 1) * P],
                                ws_sb[k][:, o * 512 : (o + 1) * 512],
                                start=False,
                                stop=(k == KT - 1),
                            )
                        yst = st_pool.tile(
                            [P, 512], f32, tag="yst", bufs=2, name=f"yst{rep}_{c-1}_{ts}_{o}"
                        )
                        nc.vector.tensor_copy(yst[:], psY[:])
                        nc.sync.dma_start(
                            y_d[t0p + ts * P : t0p + (ts + 1) * P, o * 512 : (o + 1) * 512],
                            yst[:],
                        )

                    # h back-transpose for this row-subtile
                    hst = st_pool.tile(
                        [P, D], f32, tag="hst", bufs=2, name=f"hst{rep}_{c-1}_{ts}"
                    )
                    for j4 in range(0, KT, 4):
                        pp = ps_pose.tile([P, 4 * P], f32, tag="pose", name=f"hp{rep}_{c-1}_{ts}_{j4}")
                        for i in range(4):
                            j = j4 + i
                            nc.tensor.transpose(
                                pp[:, i * P : (i + 1) * P],
                                hT_p[j // 2][:, (j % 2) * TC + ts * P : (j % 2) * TC + (ts + 1) * P],
                                ident[:],
                            )
                        nc.scalar.copy(
                            hst[:, j4 * P : (j4 + 4) * P], pp[:]
                        )
                    nc.sync.dma_start(
                        h_d[t0p + ts * P : t0p + (ts + 1) * P, :], hst[:]
                    )

            if c < NCHUNK:
                # --- a/b matmuls + tanh + scan + gelu, per j-pair ---
                hT_cur, gT_cur = [], []
                for jp in range(NP):
                    psA = ps_ab.tile([P, 2 * TC], f32, tag="ab", name=f"psA{rep}_{c}_{jp}")
                    psB = ps_ab.tile([P, 2 * TC], f32, tag="ab", name=f"psB{rep}_{c}_{jp}")
                    for i in range(2):
                        j = 2 * jp + i
                        for k in range(KT):
                            nc.tensor.matmul(
                                psA[:, i * TC : (i + 1) * TC],
                                wa_sb[k][:, j * P : (j + 1) * P],
                                xT[:, k * TC : (k + 1) * TC],
                                start=(k == 0),
                                stop=(k == KT - 1),
                            )
                        for k in range(KT):
                            nc.tensor.matmul(
                                psB[:, i * TC : (i + 1) * TC],
                                wb_sb[k][:, j * P : (j + 1) * P],
                                xT[:, k * TC : (k + 1) * TC],
                                start=(k == 0),
                                stop=(k == KT - 1),
                            )
                    aT = sc_pool.tile([P, 2 * TC], f32, tag=f"aT{jp}", bufs=1, name=f"aT{rep}_{c}_{jp}")
                    for i in range(2):
                        j = 2 * jp + i
                        # sigmoid(z+ba) = 0.5 + 0.5*tanh(0.5*z + 0.5*ba)
                        nc.scalar.activation(
                            aT[:, i * TC : (i + 1) * TC],
                            psA[:, i * TC : (i + 1) * TC],
                            AF.Tanh,
                            bias=ba_sb[j][:],
                            scale=0.5,
                        )
                    nc.gpsimd.tensor_scalar(
                        aT[:], aT[:], 0.5, 0.5, op0=ALU.mult, op1=ALU.add
                    )

                    hT = sc_pool.tile([P, 2 * TC], f32, tag=f"hT{jp}", name=f"hT{rep}_{c}_{jp}")
                    for i in range(2):
                        j = 2 * jp + i
                        init = (
                            h0_sb[j][:, 0:1]
                            if c == 0
                            else hT_prev[jp][:, (i + 1) * TC - 1 : (i + 1) * TC]
                        )
                        nc.vector.tensor_tensor_scan(
                            hT[:, i * TC : (i + 1) * TC],
                            aT[:, i * TC : (i + 1) * TC],
                            psB[:, i * TC : (i + 1) * TC],
                            init,
                            op0=ALU.mult,
                            op1=ALU.add,
                        )
                    gT = sc_pool.tile([P, 2 * TC], f32r, tag=f"gT{jp}", name=f"gT{rep}_{c}_{jp}")
                    nc.scalar.activation(gT[:], hT[:], AF.Gelu_apprx_tanh)
                    hT_cur.append(hT)
                    gT_cur.append(gT)

                if c == 0 and rep == 0:
                    load_late_weights()
                pend = (xT, gT_cur, hT_cur)
                hT_prev = hT_cur

    nc.finalize()
    return nc


def kernel(x, h0, W_a, b_a, W_b, W_y, W_skip):
    if "nc" not in _CACHE:
        _CACHE["nc"] = _build()
    nc = _CACHE["nc"]

    in_maps = []
    for b in range(B):
        in_maps.append(
            {
                "x": np.ascontiguousarray(np.asarray(x[b], dtype=np.float32)),
                "h0": np.ascontiguousarray(np.asarray(h0[b], dtype=np.float32)),
                "wa": np.ascontiguousarray(np.asarray(W_a, dtype=np.float32)),
                "ba": np.ascontiguousarray(np.asarray(b_a, dtype=np.float32)),
                "wb": np.ascontiguousarray(np.asarray(W_b, dtype=np.float32)),
                "wy": np.ascontiguousarray(np.asarray(W_y, dtype=np.float32)),
                "ws": np.ascontiguousarray(np.asarray(W_skip, dtype=np.float32)),
            }
        )

    res = run_bass_kernel_spmd(nc, in_maps, core_ids=list(range(B)))
    h = np.stack([r["h_out"] for r in res.results])
    y = np.stack([r["y_out"] for r in res.results])
    return h, y



# revision 2
# speedup vs baseline: 1.0322x; 1.0322x over previous
"""Trainium2 Bass kernel for the Memoroid linear-recurrence block.

Math (per batch b):
    a = sigmoid(x @ W_a + b_a)          [T, D]
    bm = x @ W_b                        [T, D]
    h_t = a_t * h_{t-1} + bm_t          (h_{-1} = h0, scan over t)
    y = gelu_tanh(h) @ W_y + x @ W_skip [T, D]
Returns (h, y).

Strategy: data-parallel over batch (8 sequences -> 8 cores). Per core,
work in transposed layout [d, t] so the recurrence maps onto the DVE
tensor_tensor_scan instruction (state = a*state + b along the free dim).

All matmul operands are bf16 (converted on the host, halving input DMA
and PE weight-load traffic); PSUM accumulation stays fp32. x is loaded
pre-transposed straight from HBM via the DMA xbar transpose (2-byte
dtype requirement is why x ships as bf16), so the PE spends no cycles
transposing x. The sigmoid path (psA -> tanh -> affine -> scan operand
a) stays fp32: rounding `a` to bf16 near 1.0 would perturb long-memory
channels by delta_a/(1-a) ~ O(1). The scan's internal state is fp32
regardless of output dtype; h is written out bf16 (0.4% rounding, well
inside tolerance), which also makes the PE h-transposes 1 cycle/row.
y is computed in natural [t, d] orientation directly (lhsT = gelu(h)^T
and x^T tiles), accumulated fp32 in PSUM and stored fp32.
"""

import sys

for _p in ("/opt/trn_rl_repo",):
    if _p not in sys.path:
        sys.path.insert(0, _p)

from contextlib import ExitStack

import numpy as np

import concourse.bass as bass
import concourse.bacc as bacc
import concourse.mybir as mybir
from concourse import tile
from concourse.bass_utils import run_bass_kernel_spmd
from concourse.masks import make_identity

B, T, D = 8, 4096, 1024
P = 128
KT = D // P            # 8 partition tiles along any d-dimension
TC = 512               # time-chunk length (scan tile free dim)
NCHUNK = T // TC       # 8
TS = TC // P           # 4  (128-row subtiles per chunk)
NO = D // 512          # 2  (512-wide output column chunks)

f32 = mybir.dt.float32
bf16 = mybir.dt.bfloat16

_CACHE = {}


def _build():
    nc = bacc.Bacc()

    x_d = nc.declare_dram_parameter("x", [T, D], bf16, False)
    h0_d = nc.declare_dram_parameter("h0", [D], f32, False)
    wa_d = nc.declare_dram_parameter("wa", [D, D], bf16, False)
    ba_d = nc.declare_dram_parameter("ba", [D], f32, False)
    wb_d = nc.declare_dram_parameter("wb", [D, D], bf16, False)
    wy_d = nc.declare_dram_parameter("wy", [D, D], bf16, False)
    ws_d = nc.declare_dram_parameter("ws", [D, D], bf16, False)
    h_d = nc.declare_dram_parameter("h_out", [T, D], f32, True)
    y_d = nc.declare_dram_parameter("y_out", [T, D], f32, True)

    AF = mybir.ActivationFunctionType
    ALU = mybir.AluOpType

    with tile.TileContext(nc) as tc, ExitStack() as ctx:
        wpool = ctx.enter_context(tc.tile_pool(name="weights", bufs=1))
        const_pool = ctx.enter_context(tc.tile_pool(name="const", bufs=1))
        xt_pool = ctx.enter_context(tc.tile_pool(name="xt", bufs=2))
        sc_pool = ctx.enter_context(tc.tile_pool(name="scan", bufs=2))
        st_pool = ctx.enter_context(tc.tile_pool(name="stage", bufs=1))
        ps_pose = ctx.enter_context(tc.tile_pool(name="pose", bufs=2, space="PSUM"))
        ps_ab = ctx.enter_context(tc.tile_pool(name="ab", bufs=4, space="PSUM"))
        ps_y = ctx.enter_context(tc.tile_pool(name="ypsum", bufs=2, space="PSUM"))

        ident = const_pool.tile([P, P], bf16, name="ident")
        make_identity(nc, ident[:])

        # --- chunk-0 xT + wa are the PE's first dependency -> interleave
        # their DMAs (k-paired) so the first A accumulation group can
        # start as soon as pair 0 lands.
        wa_sb, wb_sb, wy_sb, ws_sb = [], [], [], []
        xT0 = xt_pool.tile([P, KT * TC], bf16, tag="xT", name="xT0")
        for k in range(KT):
            nc.sync.dma_start(
                xT0[:, k * TC : (k + 1) * TC],
                x_d[0:TC, k * P : (k + 1) * P],
                transpose=True,
            )
            t_ = wpool.tile([P, D], bf16, tag=f"wa{k}", name=f"wa{k}")
            nc.sync.dma_start(t_[:], wa_d[k * P : (k + 1) * P, :])
            wa_sb.append(t_)

        # ba/h0 (needed by chunk 0's ACT/scan) go on the Activation HWDGE
        # queue so they don't sit behind the weight stream.
        ba_sb, h0_sb = [], []
        for j in range(KT):
            bt = const_pool.tile([P, 1], f32, tag=f"ba{j}", name=f"ba{j}")
            nc.scalar.dma_start(bt[:], ba_d[j * P : (j + 1) * P].unsqueeze(1))
            # sigmoid(z) is computed as 0.5 + 0.5*tanh(z/2) so every ACT op
            # (Tanh/Gelu_apprx_tanh/Copy) shares one activation table ->
            # no per-op table reloads. Pre-halve the bias for the tanh form.
            bh = const_pool.tile([P, 1], f32, tag=f"bah{j}", name=f"bah{j}")
            nc.scalar.mul(bh[:], bt[:], 0.5)
            ba_sb.append(bh)
            ht = const_pool.tile([P, 1], f32, tag=f"h0{j}", name=f"h0{j}")
            nc.scalar.dma_start(ht[:], h0_d[j * P : (j + 1) * P].unsqueeze(1))
            h0_sb.append(ht)

        for k in range(KT):
            t_ = wpool.tile([P, D], bf16, tag=f"wb{k}", name=f"wb{k}")
            nc.sync.dma_start(t_[:], wb_d[k * P : (k + 1) * P, :])
            wb_sb.append(t_)

        def load_late_weights():
            for k in range(KT):
                for lst, dram, nm in ((wy_sb, wy_d, "wy"), (ws_sb, ws_d, "ws")):
                    t_ = wpool.tile([P, D], bf16, tag=f"{nm}{k}", name=f"{nm}{k}")
                    nc.sync.dma_start(t_[:], dram[k * P : (k + 1) * P, :])
                    lst.append(t_)

        hT_prev = [None] * KT   # previous chunk's hT tiles (carry + Y phase)
        pend = None             # (xT, gT list, hT list) of previous chunk

        for c in range(NCHUNK + 1):
            if c < NCHUNK:
                t0 = c * TC
                if c == 0:
                    xT = xT0
                else:
                    xT = xt_pool.tile([P, KT * TC], bf16, tag="xT", name=f"xT{c}")
                    for k in range(KT):
                        nc.sync.dma_start(
                            xT[:, k * TC : (k + 1) * TC],
                            x_d[t0 : t0 + TC, k * P : (k + 1) * P],
                            transpose=True,
                        )

            if c >= 1:
                # --- phase B for chunk c-1: y matmuls + stores + h transpose ---
                xT_p, gT_p, hT_p = pend
                t0p = (c - 1) * TC
                for ts in range(TS):
                    for o in range(NO):
                        psY = ps_y.tile([P, 512], f32, tag="y", name=f"psY{c-1}_{ts}_{o}")
                        # skip-path matmuls first: they depend only on xT_p,
                        # giving the trailing scan/gelu of chunk c-1 slack
                        # before gT_p[j] is consumed.
                        for k in range(KT):
                            nc.tensor.matmul(
                                psY[:],
                                xT_p[:, k * TC + ts * P : k * TC + (ts + 1) * P],
                                ws_sb[k][:, o * 512 : (o + 1) * 512],
                                start=(k == 0),
                                stop=False,
                            )
                        for j in range(KT):
                            nc.tensor.matmul(
                                psY[:],
                                gT_p[j][:, ts * P : (ts + 1) * P],
                                wy_sb[j][:, o * 512 : (o + 1) * 512],
                                start=False,
                                stop=(j == KT - 1),
                            )
                        yst = st_pool.tile(
                            [P, 512], f32, tag="yst", bufs=3, name=f"yst{c-1}_{ts}_{o}"
                        )
                        nc.vector.tensor_copy(yst[:], psY[:])
                        nc.scalar.dma_start(
                            y_d[t0p + ts * P : t0p + (ts + 1) * P, o * 512 : (o + 1) * 512],
                            yst[:],
                        )

                for ts in range(TS):
                    pose = ps_pose.tile([P, D], bf16, tag="pose", name=f"hp{c-1}_{ts}")
                    for j in range(KT):
                        nc.tensor.transpose(
                            pose[:, j * P : (j + 1) * P],
                            hT_p[j][:, ts * P : (ts + 1) * P],
                            ident[:],
                        )
                    hst = st_pool.tile(
                        [P, D], f32, tag="hst", bufs=3, name=f"hst{c-1}_{ts}"
                    )
                    nc.scalar.copy(hst[:], pose[:])
                    nc.scalar.dma_start(
                        h_d[t0p + ts * P : t0p + (ts + 1) * P, :], hst[:]
                    )

            if c < NCHUNK:
                # --- A sweep: all 8 j accumulation groups (wa only), then
                # B sweep (wb only). Lets chunk 0 start before wb arrives.
                aT = []
                for j in range(KT):
                    psA = ps_ab.tile([P, TC], f32, tag="ab", name=f"psA{c}_{j}")
                    for k in range(KT):
                        nc.tensor.matmul(
                            psA[:],
                            wa_sb[k][:, j * P : (j + 1) * P],
                            xT[:, k * TC : (k + 1) * TC],
                            start=(k == 0),
                            stop=(k == KT - 1),
                        )
                    a_ = sc_pool.tile([P, TC], f32, tag=f"aT{j}", bufs=1, name=f"aT{c}_{j}")
                    # sigmoid(z+ba) = 0.5 + 0.5*tanh(0.5*z + 0.5*ba)
                    nc.scalar.activation(
                        a_[:], psA[:], AF.Tanh, bias=ba_sb[j][:], scale=0.5
                    )
                    nc.gpsimd.tensor_scalar(
                        a_[:], a_[:], 0.5, 0.5, op0=ALU.mult, op1=ALU.add
                    )
                    aT.append(a_)

                hT_cur, gT_cur = [], []
                for j in range(KT):
                    psB = ps_ab.tile([P, TC], f32, tag="ab", name=f"psB{c}_{j}")
                    for k in range(KT):
                        nc.tensor.matmul(
                            psB[:],
                            wb_sb[k][:, j * P : (j + 1) * P],
                            xT[:, k * TC : (k + 1) * TC],
                            start=(k == 0),
                            stop=(k == KT - 1),
                        )
                    hT = sc_pool.tile([P, TC], bf16, tag=f"hT{j}", name=f"hT{c}_{j}")
                    init = (
                        h0_sb[j][:, 0:1]
                        if c == 0
                        else hT_prev[j][:, TC - 1 : TC]
                    )
                    nc.vector.tensor_tensor_scan(
                        hT[:],
                        aT[j][:],
                        psB[:],
                        init,
                        op0=ALU.mult,
                        op1=ALU.add,
                    )
                    gT = sc_pool.tile([P, TC], bf16, tag=f"gT{j}", name=f"gT{c}_{j}")
                    nc.scalar.activation(gT[:], hT[:], AF.Gelu_apprx_tanh)
                    hT_cur.append(hT)
                    gT_cur.append(gT)

                if c == 0:
                    load_late_weights()
                pend = (xT, gT_cur, hT_cur)
                hT_prev = hT_cur

    nc.finalize()
    return nc


def kernel(x, h0, W_a, b_a, W_b, W_y, W_skip):
    import ml_dtypes

    bf = ml_dtypes.bfloat16

    if "nc" not in _CACHE:
        _CACHE["nc"] = _build()
    nc = _CACHE["nc"]

    wa = np.ascontiguousarray(np.asarray(W_a, dtype=np.float32).astype(bf))
    wb = np.ascontiguousarray(np.asarray(W_b, dtype=np.float32).astype(bf))
    wy = np.ascontiguousarray(np.asarray(W_y, dtype=np.float32).astype(bf))
    ws = np.ascontiguousarray(np.asarray(W_skip, dtype=np.float32).astype(bf))
    ba = np.ascontiguousarray(np.asarray(b_a, dtype=np.float32))

    in_maps = []
    for b in range(B):
        in_maps.append(
            {
                "x": np.ascontiguousarray(
                    np.asarray(x[b], dtype=np.float32).astype(bf)
                ),
                "h0": np.ascontiguousarray(np.asarray(h0[b], dtype=np.float32)),
                "wa": wa,
                "ba": ba,
                "wb": wb,
                "wy": wy,
                "ws": ws,
            }
        )

    res = run_bass_kernel_spmd(nc, in_maps, core_ids=list(range(B)))
    h = np.stack([r["h_out"] for r in res.results])
    y = np.stack([r["y_out"] for r in res.results])
    return h, y
